# revision 1
# baseline (speedup 1.0000x reference)
"""BinaryConv2d (3x3, SAME, NHWC) Trainium2 Bass kernel.

Strategy:
  - Data-parallel over batch: 32 images -> 8 cores x 4 images. Weights/bias
    replicated. No collectives needed.
  - Host prep (tiny): Wq = sign(W) cast to bf16 (+-1 exact), laid out as
    [cin, 9, cout]; bias replicated to [128, cout] f32.
  - Per core, per image (pipelined in 16-row chunks; casts are explicitly
    paced behind transposes so the SDMA fabric never starves the PE's
    critical path):
      1. SWDGE cast-DMA: x rows f32 NHWC (HBM) -> bf16 [16, W+2, cin] HBM
         scratch slices; pad columns 0 and W+1 zeroed from a zero SBUF tile
         (left/right SAME pads).
      2. HWDGE xbar transpose-DMA per chunk: scratch [(16*(W+2)), cin] ->
         SBUF chunk tile [cin, 16*(W+2)], channel-major (contiguous dst --
         the xbar scrambles strided or non-32B-aligned destinations).
      3. For each output row r: accumulate 9 (clipped at top/bottom) matmuls
         into PSUM [W, cout]: lhsT = chunk[:, (row_off)*(W+2)+dw : +W]
         (stationary, pixels on PSUM partitions), rhs = Wq[:, 3*dh+dw, :]
         (streaming, cout free dim). fp32 PSUM accumulation, bf16 operands
         (rel err ~1.7e-3 vs the f32 reference).
      4. DVE tensor_add(psum, bias) -> SBUF f32 staging [W, 4, cout]; one
         HWDGE DMA per 4 rows out to NHWC HBM (keeps HWDGE op count low --
         per-DMA issue is ~0.7us and queue-pacing semaphores couple all
         HWDGE queues).

Image 0's first chunk is split 16 -> 8+8 rows to halve the cold-start
cast->transpose->matmul critical path; everything else uses 16-row chunks.

Measured on 8 axon-tunneled TRN2 cores: 466-482 us HW exec across runs
(466.0 us with the split first chunk); matmul stream sustains ~110 ns per
N=256 matmul (2.4 GHz warm, LDWEIGHTS hidden by the PE reorder window),
which is its issue-rate floor.
"""

import numpy as np

N_CORES = 8
H = 112
W_DIM = 112
CIN = 128
COUT = 256
BATCH = 32
IMG_PER_CORE = BATCH // N_CORES


def _build_program(n_img, h, w, cin, cout):
    import bass_rust
    import concourse.bacc as bacc
    import concourse.mybir as mybir
    import concourse.tile as tile

    f32 = mybir.dt.float32
    bf16 = mybir.dt.bfloat16

    nc = bacc.Bacc(
        "TRN2", target_bir_lowering=False, debug=False, num_devices=N_CORES
    )
    x_d = nc.dram_tensor("x", [n_img, h, w, cin], f32, kind="ExternalInput").ap()
    w_d = nc.dram_tensor("w", [cin, 9, cout], bf16, kind="ExternalInput").ap()
    b_d = nc.dram_tensor("b", [128, cout], f32, kind="ExternalInput").ap()
    out_d = nc.dram_tensor(
        "out", [n_img, h, w, cout], f32, kind="ExternalOutput"
    ).ap()

    wp = w + 2  # padded row width in the transposed SBUF image
    rc = 16  # rows per cast/transpose chunk; (rc * wp) % 16 == 0 required
    assert h % rc == 0 and (rc * wp) % 16 == 0
    n_chunks = h // rc
    # image 0 only: split the first chunk 16 -> 8+8 so the very first
    # cast->transpose->matmul critical path is half as long (sz=8 keeps
    # (sz*wp) % 16 == 0); later images are prefetched anyway
    sizes_by_img = []
    for img in range(n_img):
        if img == 0 and h >= 32:
            sizes_by_img.append([8, 8] + [16] * ((h - 16) // 16))
        else:
            sizes_by_img.append([16] * (h // 16))
    rowmaps = []
    for sizes in sizes_by_img:
        starts = [sum(sizes[:i]) for i in range(len(sizes))]
        rm = {}
        for ci, (s0, sz) in enumerate(zip(starts, sizes)):
            for i in range(s0, s0 + sz):
                rm[i] = (ci, i - s0)
        rowmaps.append((sizes, starts, rm))
    ob = 4  # output rows batched per store DMA
    assert h % ob == 0

    with tile.TileContext(nc) as tc:
        with (
            tc.tile_pool(name="consts", bufs=1) as cpool,
            tc.tile_pool(name="scratch", bufs=n_img, space="DRAM") as dpool,
            tc.tile_pool(name="xt", bufs=sum(len(s[0]) for s in rowmaps))
            as xtpool,
            tc.tile_pool(name="psum", bufs=8, space="PSUM") as pspool,
            tc.tile_pool(name="outs", bufs=8) as opool,
        ):
            w_t = cpool.tile([cin, 9, cout], bf16)
            nc.sync.dma_start(out=w_t[:], in_=w_d[:])
            b_t = cpool.tile([128, cout], f32)
            nc.sync.dma_start(out=b_t[:], in_=b_d[:])
            zt = cpool.tile([h, cin], bf16)
            nc.vector.memset(zt[:], 0.0)

            # per-image DRAM scratch [h, wp, cin]; pad cols zeroed once per
            # image (SWDGE, keeping the HWDGE queue free for transposes);
            # casts fill 16-row slices; transposes lift 16-row slices to
            # SBUF chunk tiles [cin, rc*wp], channel-major.
            chunks = [[None] * len(rowmaps[g][0]) for g in range(n_img)]
            transpose_insts = []
            PACE = 3  # cast for chunk g waits on transpose g-PACE: keeps the
            # SDMA fabric from flooding with casts and starving the
            # transposes the PE is actually waiting for

            def prep_image(img):
                sizes, starts, _rm = rowmaps[img]
                scr = dpool.tile([h, wp, cin], bf16, tag="scr")
                for c, (r0, sz) in enumerate(zip(starts, sizes)):
                    # f32 -> bf16 cast during DMA (SWDGE only)
                    cast = nc.gpsimd.dma_start(
                        out=scr[r0 : r0 + sz, 1 : w + 1, :],
                        in_=x_d[img, r0 : r0 + sz],
                    )
                    if c == 0:
                        # pad-col zeroing rides behind the first cast so the
                        # critical-path cast issues immediately
                        nc.gpsimd.dma_start(out=scr[:, 0, :], in_=zt[:])
                        nc.gpsimd.dma_start(out=scr[:, wp - 1, :], in_=zt[:])
                    g = len(transpose_insts)
                    if g >= PACE:
                        bass_rust.add_dep_helper(
                            cast.ins,
                            transpose_insts[g - PACE].ins,
                            sync=True,
                            reason="pace casts behind transposes",
                        )
                    xt = xtpool.tile([cin, sz * wp], bf16, tag="xt")
                    tr = nc.sync.dma_start(
                        out=xt[:],
                        in_=scr[r0 : r0 + sz].rearrange("a b c -> (a b) c"),
                        transpose=True,
                    )
                    transpose_insts.append(tr)
                    chunks[img][c] = xt

            def get_row(img, i):
                # lhsT base AP for input row i of image img
                ci, off = rowmaps[img][2][i]
                return chunks[img][ci], off * wp

            # issue ALL input prep up front: per-chunk region deps let
            # matmuls start as soon as chunk 0 is transposed, while the rest
            # streams in behind.
            for img in range(n_img):
                prep_image(img)

            for img in range(n_img):
                for rb in range(h // ob):
                    ot = opool.tile([w, ob, cout], f32)
                    for j in range(ob):
                        r = rb * ob + j
                        ps = pspool.tile([w, cout], f32)
                        taps = [
                            (dh, dw)
                            for dh in (0, 1, 2)
                            for dw in (0, 1, 2)
                            if 0 <= r + dh - 1 < h
                        ]
                        last = len(taps) - 1
                        for k, (dh, dw) in enumerate(taps):
                            xt, base = get_row(img, r + dh - 1)
                            nc.tensor.matmul(
                                ps[:],
                                xt[:, base + dw : base + dw + w],
                                w_t[:, 3 * dh + dw, :],
                                start=(k == 0),
                                stop=(k == last),
                            )
                        nc.vector.tensor_add(ot[:, j, :], ps[:], b_t[:w, :])
                    nc.scalar.dma_start(
                        out=out_d[img, rb * ob : (rb + 1) * ob].rearrange(
                            "j w c -> w j c"
                        ),
                        in_=ot[:],
                    )

    nc.compile()
    return nc


_cached_nc = None


def _get_program():
    global _cached_nc
    if _cached_nc is None:
        _cached_nc = _build_program(IMG_PER_CORE, H, W_DIM, CIN, COUT)
    return _cached_nc


def _prep_inputs(x, W, b):
    import ml_dtypes

    # sign with sign(0)=0, matching jnp.sign; bf16 holds +-1/0 exactly
    wq = np.sign(W.astype(np.float32)).astype(ml_dtypes.bfloat16)
    # [3,3,cin,cout] -> [cin, 9, cout]
    wq = np.ascontiguousarray(wq.transpose(2, 0, 1, 3).reshape(CIN, 9, COUT))
    b_rep = np.ascontiguousarray(
        np.broadcast_to(b.astype(np.float32), (128, COUT))
    )
    in_maps = []
    for c in range(N_CORES):
        xs = np.ascontiguousarray(
            x[c * IMG_PER_CORE : (c + 1) * IMG_PER_CORE].astype(np.float32)
        )
        in_maps.append({"x": xs, "w": wq, "b": b_rep})
    return in_maps


def run(x, W, b, trace=False, tmpdir=None):
    from concourse import bass_utils

    if trace:
        # the agent image's antenv lacks axon_hooks; wire the NTFF profile
        # hook up manually so trace=True yields exec_time_ns + pftrace
        import sys, types

        if "antenv.axon_hooks" not in sys.modules:
            import antenv
            from trn_agent_boot.trn_boot import _ntff_profile_via_ctypes

            mod = types.ModuleType("antenv.axon_hooks")
            _hook = _ntff_profile_via_ctypes("/opt/axon/libaxon_pjrt.so")
            mod.get_axon_ntff_profile_hook = lambda: _hook
            sys.modules["antenv.axon_hooks"] = mod
            antenv.axon_hooks = mod

    nc = _get_program()
    in_maps = _prep_inputs(x, W, b)
    res = bass_utils.run_bass_kernel_spmd(
        nc, in_maps, list(range(N_CORES)), trace=trace, tmpdir=tmpdir
    )
    out = np.concatenate([res.results[i]["out"] for i in range(N_CORES)], axis=0)
    return out, res


def kernel(x, W, b):
    out, _ = run(x, W, b, trace=False)
    return out



# revision 11
# speedup vs baseline: 1.0080x; 1.0080x over previous
"""BinaryConv2d (3x3, SAME, NHWC) Trainium2 Bass kernel.

Strategy:
  - Data-parallel over batch: 32 images -> 8 cores x 4 images. Weights/bias
    replicated. No collectives needed.
  - Host prep (tiny): Wq = sign(W) cast to bf16 (+-1 exact), laid out as
    [cin, 9, cout]; bias replicated to [128, cout] f32.
  - The image lives in SBUF channel-major as [cin, 128*114] bf16: rows
    0-7 and 120-127 are zero guard blocks (top/bottom SAME padding + tail
    slack), data rows 0..111 sit at block rows 8..119, each row 114 wide
    (zero pad cols 0 and 113 for left/right SAME padding).
  - Output is computed in M=128 windows over the PADDED linear pixel space
    p = r*114 + c (100 windows of 128 px per image, 12800 px incl ~2%
    garbage at c>=112 and the tail). For tap (dh, dw) the stationary
    lhsT is the contiguous 128-px slice at offset p + (dh-1)*114 +
    (dw-1) (+ guard offset) -- a plain 1-free-dim AP with full M=128 PE
    width -- and rhs = Wq[:, 3*dh+dw, :] streams cout. 9 taps accumulate
    in PSUM [128, cout] f32.
  - DVE tensor_add(psum, bias) packs 5 windows into staging [128, 5,
    cout]; one store DMA per 5 windows writes the padded-linear output
    [img, 12800, cout] f32 (20 stores/img, 640KB each, alternating
    scalar/sync HWDGE queues). The host strips pad cols: reshape
    [112, 114, 256][:, :112].
  - Input pipeline (per image, 16-row chunks, casts paced behind
    transposes): SWDGE cast-DMA x f32 NHWC -> bf16 [rows, 114, cin] HBM
    scratch (pad cols zeroed); HWDGE xbar transpose-DMA lifts 16-row
    chunks to the SBUF image tile at 32B-aligned offsets ((8+r)*228B,
    r % 8 == 0). Image 0's first chunk is split 8+8 to halve the
    cold-start critical path.
"""

import numpy as np

N_CORES = 8
H = 112
W_DIM = 112
CIN = 128
COUT = 256
BATCH = 32
IMG_PER_CORE = BATCH // N_CORES

WP = 114  # padded row width
NPX = 12800  # padded linear out px per image (112*114=12768, padded to 100 windows)
GUARD = 8  # guard rows above/below data in the SBUF image tile


def _build_program(n_img, h, w, cin, cout):
    import bass_rust
    import concourse.bacc as bacc
    import concourse.mybir as mybir
    import concourse.tile as tile

    f32 = mybir.dt.float32
    bf16 = mybir.dt.bfloat16

    nc = bacc.Bacc(
        "TRN2", target_bir_lowering=False, debug=False, num_devices=N_CORES
    )
    x_d = nc.dram_tensor("x", [n_img, h, w, cin], f32, kind="ExternalInput").ap()
    w_d = nc.dram_tensor("w", [cin, 9, cout], bf16, kind="ExternalInput").ap()
    b_d = nc.dram_tensor("b", [128, cout], f32, kind="ExternalInput").ap()
    out_d = nc.dram_tensor(
        "out", [n_img, NPX, cout], f32, kind="ExternalOutput"
    ).ap()

    wp = WP
    n_win = NPX // 128  # 100 windows of 128 px
    SG = 5  # windows batched per store DMA
    assert n_win % SG == 0
    tile_rows = GUARD + h + GUARD  # 128
    base = GUARD * wp  # SBUF px offset of data row 0

    rc = 16  # rows per cast/transpose chunk; (rc * wp) % 16 == 0 required
    assert h % rc == 0 and (rc * wp) % 16 == 0
    sizes_by_img = []
    for img in range(n_img):
        if img == 0 and h >= 32:
            sizes_by_img.append([8, 8] + [16] * ((h - 16) // 16))
        else:
            sizes_by_img.append([16] * (h // 16))

    with tile.TileContext(nc) as tc:
        with (
            tc.tile_pool(name="consts", bufs=1) as cpool,
            tc.tile_pool(name="scratch", bufs=n_img, space="DRAM") as dpool,
            tc.tile_pool(name="ximg", bufs=n_img) as xpool,
            tc.tile_pool(name="psum", bufs=8, space="PSUM") as pspool,
            tc.tile_pool(name="outs", bufs=4) as opool,
        ):
            w_t = cpool.tile([cin, 9, cout], bf16)
            nc.sync.dma_start(out=w_t[:], in_=w_d[:])
            b_t = cpool.tile([128, cout], f32)
            nc.sync.dma_start(out=b_t[:], in_=b_d[:])
            zt = cpool.tile([h, cin], bf16)
            nc.vector.memset(zt[:], 0.0)

            imgs = [None] * n_img
            transpose_insts = []
            PACE = 3  # cast for chunk g waits on transpose g-PACE

            def prep_image(img):
                it = xpool.tile([cin, tile_rows * wp], bf16, tag="ximg")
                imgs[img] = it
                # zero guard blocks (top/bottom SAME padding + tail slack)
                nc.vector.memset(it[:, 0:base], 0.0)
                nc.vector.memset(it[:, base + h * wp :], 0.0)
                scr = dpool.tile([h, wp, cin], bf16, tag="scr")
                r0 = 0
                for c, sz in enumerate(sizes_by_img[img]):
                    # f32 -> bf16 cast during DMA (SWDGE only)
                    cast = nc.gpsimd.dma_start(
                        out=scr[r0 : r0 + sz, 1 : w + 1, :],
                        in_=x_d[img, r0 : r0 + sz],
                    )
                    if c == 0:
                        # pad-col zeroing rides behind the first cast
                        nc.gpsimd.dma_start(out=scr[:, 0, :], in_=zt[:])
                        nc.gpsimd.dma_start(out=scr[:, wp - 1, :], in_=zt[:])
                    g = len(transpose_insts)
                    if g >= PACE:
                        bass_rust.add_dep_helper(
                            cast.ins,
                            transpose_insts[g - PACE].ins,
                            sync=True,
                            reason="pace casts behind transposes",
                        )
                    tr = nc.sync.dma_start(
                        out=it[:, base + r0 * wp : base + (r0 + sz) * wp],
                        in_=scr[r0 : r0 + sz].rearrange("a b c -> (a b) c"),
                        transpose=True,
                    )
                    transpose_insts.append(tr)
                    r0 += sz

            # issue ALL input prep up front: per-chunk region deps let
            # matmuls start as soon as chunk 0 is transposed, while the
            # rest streams in behind.
            for img in range(n_img):
                prep_image(img)

            TAPS = [(dh, dw) for dh in (0, 1, 2) for dw in (0, 1, 2)]
            si = 0
            for img in range(n_img):
                it = imgs[img]
                for wg in range(n_win // SG):
                    ot = opool.tile([128, SG, cout], f32)
                    for g in range(SG):
                        p0 = (wg * SG + g) * 128
                        ps = pspool.tile([128, cout], f32)
                        for k, (dh, dw) in enumerate(TAPS):
                            off = base + p0 + (dh - 1) * wp + dw
                            nc.tensor.matmul(
                                ps[:],
                                it[:, off : off + 128],
                                w_t[:, 3 * dh + dw, :],
                                start=(k == 0),
                                stop=(k == 8),
                            )
                        nc.vector.tensor_add(ot[:, g, :], ps[:], b_t[:])
                    eng = nc.scalar if si % 2 == 0 else nc.sync
                    eng.dma_start(
                        out=out_d[img, wg * SG * 128 : (wg + 1) * SG * 128]
                        .rearrange("(g p) ch -> p g ch", g=SG),
                        in_=ot[:],
                    )
                    si += 1

    nc.compile()
    return nc


_cached_nc = None


def _get_program():
    global _cached_nc
    if _cached_nc is None:
        _cached_nc = _build_program(IMG_PER_CORE, H, W_DIM, CIN, COUT)
    return _cached_nc


def _prep_inputs(x, W, b):
    import ml_dtypes

    # sign with sign(0)=0, matching jnp.sign; bf16 holds +-1/0 exactly
    wq = np.sign(W.astype(np.float32)).astype(ml_dtypes.bfloat16)
    # [3,3,cin,cout] -> [cin, 9, cout]
    wq = np.ascontiguousarray(wq.transpose(2, 0, 1, 3).reshape(CIN, 9, COUT))
    b_rep = np.ascontiguousarray(
        np.broadcast_to(b.astype(np.float32), (128, COUT))
    )
    in_maps = []
    for c in range(N_CORES):
        xs = np.ascontiguousarray(
            x[c * IMG_PER_CORE : (c + 1) * IMG_PER_CORE].astype(np.float32)
        )
        in_maps.append({"x": xs, "w": wq, "b": b_rep})
    return in_maps


def run(x, W, b, trace=False, tmpdir=None):
    from concourse import bass_utils

    if trace:
        # the agent image's antenv lacks axon_hooks; wire the NTFF profile
        # hook up manually so trace=True yields exec_time_ns + pftrace
        import sys, types

        if "antenv.axon_hooks" not in sys.modules:
            import antenv
            from trn_agent_boot.trn_boot import _ntff_profile_via_ctypes

            mod = types.ModuleType("antenv.axon_hooks")
            _hook = _ntff_profile_via_ctypes("/opt/axon/libaxon_pjrt.so")
            mod.get_axon_ntff_profile_hook = lambda: _hook
            sys.modules["antenv.axon_hooks"] = mod
            antenv.axon_hooks = mod

    nc = _get_program()
    in_maps = _prep_inputs(x, W, b)
    res = bass_utils.run_bass_kernel_spmd(
        nc, in_maps, list(range(N_CORES)), trace=trace, tmpdir=tmpdir
    )
    # device output is padded-linear [n_img, 12800, cout]; strip the pad
    # cols (c=112,113) and the tail on the host
    outs = []
    for i in range(N_CORES):
        o = res.results[i]["out"][:, : H * WP, :].reshape(
            IMG_PER_CORE, H, WP, COUT
        )[:, :, :W_DIM, :]
        outs.append(o)
    out = np.ascontiguousarray(np.concatenate(outs, axis=0))
    return out, res


def kernel(x, W, b):
    out, _ = run(x, W, b, trace=False)
    return out


# revision 12
# speedup vs baseline: 1.0825x; 1.0740x over previous
"""BinaryConv2d (3x3, SAME, NHWC) Trainium2 Bass kernel.

Strategy:
  - Data-parallel over batch: 32 images -> 8 cores x 4 images. Weights/bias
    replicated. No collectives needed.
  - Host prep (tiny): Wq = sign(W) cast to bf16 (+-1 exact), laid out as
    [cin, 9, cout]; bias replicated to [128, cout] f32.
  - The image lives in SBUF channel-major as [cin, 128*114] bf16: rows
    0-7 and 120-127 are zero guard blocks (top/bottom SAME padding + tail
    slack), data rows 0..111 sit at block rows 8..119, each row 114 wide
    (zero pad cols 0 and 113 for left/right SAME padding).
  - Output is computed in M=128 windows over the PADDED linear pixel space
    p = r*114 + c (100 windows of 128 px per image, 12800 px incl ~2%
    garbage at c>=112 and the tail). For tap (dh, dw) the stationary
    lhsT is the contiguous 128-px slice at offset p + (dh-1)*114 +
    (dw-1) (+ guard offset) -- a plain 1-free-dim AP with full M=128 PE
    width -- and rhs = Wq[:, 3*dh+dw, :] streams cout. 9 taps accumulate
    in PSUM [128, cout] f32.
  - DVE tensor_add(psum, bias) packs 5 windows into staging [128, 5,
    cout]; one store DMA per 5 windows writes the padded-linear output
    [img, 12800, cout] f32 (20 stores/img, 640KB each, alternating
    scalar/sync HWDGE queues). The host strips pad cols: reshape
    [112, 114, 256][:, :112].
  - Input pipeline (per image, 16-row chunks, casts paced behind
    transposes): SWDGE cast-DMA x f32 NHWC -> bf16 [rows, 114, cin] HBM
    scratch (pad cols zeroed); HWDGE xbar transpose-DMA lifts 16-row
    chunks to the SBUF image tile at 32B-aligned offsets ((8+r)*228B,
    r % 8 == 0). Image 0's first chunk is split 8+8 to halve the
    cold-start critical path.
"""

import numpy as np

N_CORES = 8
H = 112
W_DIM = 112
CIN = 128
COUT = 256
BATCH = 32
IMG_PER_CORE = BATCH // N_CORES

WP = 114  # padded row width
NPX = 12800  # padded linear out px per image (112*114=12768, padded to 100 windows)
GUARD = 8  # guard rows above/below data in the SBUF image tile


def _build_program(n_img, h, w, cin, cout):
    import bass_rust
    import concourse.bacc as bacc
    import concourse.mybir as mybir
    import concourse.tile as tile

    f32 = mybir.dt.float32
    bf16 = mybir.dt.bfloat16

    nc = bacc.Bacc(
        "TRN2", target_bir_lowering=False, debug=False, num_devices=N_CORES
    )
    x_d = nc.dram_tensor("x", [n_img, h, w, cin], f32, kind="ExternalInput").ap()
    w_d = nc.dram_tensor("w", [cin, 9, cout], bf16, kind="ExternalInput").ap()
    b_d = nc.dram_tensor("b", [128, cout], f32, kind="ExternalInput").ap()
    out_d = nc.dram_tensor(
        "out", [n_img, NPX, cout], f32, kind="ExternalOutput"
    ).ap()

    wp = WP
    n_win = NPX // 128  # 100 windows of 128 px
    SG = 5  # windows batched per store DMA
    assert n_win % SG == 0
    tile_rows = GUARD + h + GUARD  # 128
    base = GUARD * wp  # SBUF px offset of data row 0

    rc = 16  # rows per cast/transpose chunk; (rc * wp) % 16 == 0 required
    assert h % rc == 0 and (rc * wp) % 16 == 0
    sizes_by_img = []
    for img in range(n_img):
        if img == 0 and h >= 32:
            sizes_by_img.append([8, 8] + [16] * ((h - 16) // 16))
        else:
            sizes_by_img.append([16] * (h // 16))

    with tile.TileContext(nc) as tc:
        with (
            tc.tile_pool(name="consts", bufs=1) as cpool,
            tc.tile_pool(name="scratch", bufs=n_img, space="DRAM") as dpool,
            tc.tile_pool(name="ximg", bufs=n_img) as xpool,
            tc.tile_pool(name="psum", bufs=8, space="PSUM") as pspool,
            tc.tile_pool(name="outs", bufs=4) as opool,
        ):
            w_t = cpool.tile([cin, 9, cout], bf16)
            nc.sync.dma_start(out=w_t[:], in_=w_d[:])
            b_t = cpool.tile([128, cout], f32)
            nc.sync.dma_start(out=b_t[:], in_=b_d[:])
            zt = cpool.tile([h, cin], bf16)
            nc.vector.memset(zt[:], 0.0)

            imgs = [None] * n_img
            transpose_insts = []
            PACE = 3  # cast for chunk g waits on transpose g-PACE

            def prep_image(img):
                it = xpool.tile([cin, tile_rows * wp], bf16, tag="ximg")
                imgs[img] = it
                # zero guard blocks (top/bottom SAME padding + tail slack)
                nc.vector.memset(it[:, 0:base], 0.0)
                nc.vector.memset(it[:, base + h * wp :], 0.0)
                scr = dpool.tile([h, wp, cin], bf16, tag="scr")
                r0 = 0
                for c, sz in enumerate(sizes_by_img[img]):
                    # f32 -> bf16 cast during DMA (SWDGE only)
                    cast = nc.gpsimd.dma_start(
                        out=scr[r0 : r0 + sz, 1 : w + 1, :],
                        in_=x_d[img, r0 : r0 + sz],
                    )
                    if c == 0:
                        # pad-col zeroing rides behind the first cast
                        nc.gpsimd.dma_start(out=scr[:, 0, :], in_=zt[:])
                        nc.gpsimd.dma_start(out=scr[:, wp - 1, :], in_=zt[:])
                    g = len(transpose_insts)
                    if g >= PACE:
                        bass_rust.add_dep_helper(
                            cast.ins,
                            transpose_insts[g - PACE].ins,
                            sync=True,
                            reason="pace casts behind transposes",
                        )
                    tr = nc.sync.dma_start(
                        out=it[:, base + r0 * wp : base + (r0 + sz) * wp],
                        in_=scr[r0 : r0 + sz].rearrange("a b c -> (a b) c"),
                        transpose=True,
                    )
                    transpose_insts.append(tr)
                    r0 += sz

            # issue ALL input prep up front: per-chunk region deps let
            # matmuls start as soon as chunk 0 is transposed, while the
            # rest streams in behind.
            for img in range(n_img):
                prep_image(img)

            TAPS = [(dh, dw) for dh in (0, 1, 2) for dw in (0, 1, 2)]
            si = 0
            for img in range(n_img):
                it = imgs[img]
                for wg in range(n_win // SG):
                    ot = opool.tile([128, SG, cout], f32)
                    for g in range(SG):
                        p0 = (wg * SG + g) * 128
                        ps = pspool.tile([128, cout], f32)
                        for k, (dh, dw) in enumerate(TAPS):
                            off = base + p0 + (dh - 1) * wp + dw
                            nc.tensor.matmul(
                                ps[:],
                                it[:, off : off + 128],
                                w_t[:, 3 * dh + dw, :],
                                start=(k == 0),
                                stop=(k == 8),
                            )
                        nc.vector.tensor_add(ot[:, g, :], ps[:], b_t[:])
                    # all stores on the scalar HWDGE queue: the sync queue
                    # carries the input transposes the PE is waiting on
                    nc.scalar.dma_start(
                        out=out_d[img, wg * SG * 128 : (wg + 1) * SG * 128]
                        .rearrange("(g p) ch -> p g ch", g=SG),
                        in_=ot[:],
                    )
                    si += 1

    nc.compile()
    return nc


_cached_nc = None


def _get_program():
    global _cached_nc
    if _cached_nc is None:
        _cached_nc = _build_program(IMG_PER_CORE, H, W_DIM, CIN, COUT)
    return _cached_nc


def _prep_inputs(x, W, b):
    import ml_dtypes

    # sign with sign(0)=0, matching jnp.sign; bf16 holds +-1/0 exactly
    wq = np.sign(W.astype(np.float32)).astype(ml_dtypes.bfloat16)
    # [3,3,cin,cout] -> [cin, 9, cout]
    wq = np.ascontiguousarray(wq.transpose(2, 0, 1, 3).reshape(CIN, 9, COUT))
    b_rep = np.ascontiguousarray(
        np.broadcast_to(b.astype(np.float32), (128, COUT))
    )
    in_maps = []
    for c in range(N_CORES):
        xs = np.ascontiguousarray(
            x[c * IMG_PER_CORE : (c + 1) * IMG_PER_CORE].astype(np.float32)
        )
        in_maps.append({"x": xs, "w": wq, "b": b_rep})
    return in_maps


def run(x, W, b, trace=False, tmpdir=None):
    from concourse import bass_utils

    if trace:
        # the agent image's antenv lacks axon_hooks; wire the NTFF profile
        # hook up manually so trace=True yields exec_time_ns + pftrace
        import sys, types

        if "antenv.axon_hooks" not in sys.modules:
            import antenv
            from trn_agent_boot.trn_boot import _ntff_profile_via_ctypes

            mod = types.ModuleType("antenv.axon_hooks")
            _hook = _ntff_profile_via_ctypes("/opt/axon/libaxon_pjrt.so")
            mod.get_axon_ntff_profile_hook = lambda: _hook
            sys.modules["antenv.axon_hooks"] = mod
            antenv.axon_hooks = mod

    nc = _get_program()
    in_maps = _prep_inputs(x, W, b)
    res = bass_utils.run_bass_kernel_spmd(
        nc, in_maps, list(range(N_CORES)), trace=trace, tmpdir=tmpdir
    )
    # device output is padded-linear [n_img, 12800, cout]; strip the pad
    # cols (c=112,113) and the tail on the host
    outs = []
    for i in range(N_CORES):
        o = res.results[i]["out"][:, : H * WP, :].reshape(
            IMG_PER_CORE, H, WP, COUT
        )[:, :, :W_DIM, :]
        outs.append(o)
    out = np.ascontiguousarray(np.concatenate(outs, axis=0))
    return out, res


def kernel(x, W, b):
    out, _ = run(x, W, b, trace=False)
    return out
